# revision 28
# baseline (speedup 1.0000x reference)
"""Trainium2 Bass kernel for nn_BiasBlock (gnn_message_passing).

Computes, for N=100k nodes / E=640k edges / C=128 channels:
    h  = synth1(x)   -> synth2(h)            (modulated linears, LeakyReLU/identity)
    agg = segment_sum(el_W[src], dst) + el_b -> synth3(agg)
    y  = leaky_relu(h + agg, 0.01)

Strategy: shard nodes across 8 NeuronCores (12500 each, padded to 12544).
Per core, activations live transposed ([channel, node]). Host folds the third
modulated weight into the edge weights (z = el_W @ Wm3.T), so gathered edge
rows accumulate straight into the main PSUM accumulator: per 512-node
super-tile, ps = Wm2 @ h1 (start=True) then one staircase matmul per 128-edge
chunk (start=False) adds the whole edge branch. All per-node constants
(lin2/el2 noise, biases, el_b @ Wm3.T) are merged host-side into one fp8
stream; x stays fp16; the x/noise streams are byte-packed so each super-tile
needs a single DMA. Staircase matrices ship as fp8 (exact 0/1; matmul takes
mixed fp16 lhsT x fp8 rhs), halving their traffic. Gather indices ship
host-replicated ([128, n/16] int16) with each bank's first-batch columns as
their own small DMA so the first gathers start ~10us earlier. Emission is
software-pipelined three stages deep (stream/stair DMA 2 super-tiles ahead,
mm1 1 ahead) so neither the PE queue nor the gather queues ever stall on the
x-branch. The SWDGE descriptor-generation chain (4 queues, ~1.5us fixed +
~7.3ns/row per 1024-row gather per queue) is the kernel's critical path.

Edge branch: edges are grouped per (super-tile of 512 destination nodes) x
(z bank of 25000 rows; 4 banks keep gather indices within int16), sorted by
destination within each section, and padded to chunks of 128. z rows are
fetched in fp16 (256B) with batched gpsimd.dma_gather (1024 rows/instruction)
round-robined over all 4 SWDGE queues. Each 128-edge chunk is segment-summed
into its super-tile's [128, 512] PSUM accumulator by ONE matmul against a
static staircase matrix (edge slot -> destination column, built on the host).
"""
import os
import sys
import types

import numpy as np
import ml_dtypes

F8 = ml_dtypes.float8_e4m3  # TRN fp8e4 (E4M3 with inf, max normal 240)

# --- environment bootstrap (self-contained: no sibling imports) -------------
if "/opt/trn_rl_repo" not in sys.path:
    sys.path.insert(0, "/opt/trn_rl_repo")

_hook = {"h": None}


def _install_axon_hooks():
    """Provide antenv.axon_hooks (absent in this image) so trace=True works."""
    try:
        import antenv
    except ImportError:
        return
    if "antenv.axon_hooks" in sys.modules:
        return
    mod = types.ModuleType("antenv.axon_hooks")
    mod.set_axon_ntff_profile_hook = lambda h: _hook.__setitem__("h", h)
    mod.get_axon_ntff_profile_hook = lambda: _hook["h"]
    sys.modules["antenv.axon_hooks"] = mod
    antenv.axon_hooks = mod
    try:
        from trn_agent_boot.trn_boot import _ntff_profile_via_ctypes

        mod.set_axon_ntff_profile_hook(
            _ntff_profile_via_ctypes("/opt/axon/libaxon_pjrt.so")
        )
    except Exception:
        pass


_install_axon_hooks()

import concourse.bass_utils as _bu

_bu.upload_artifacts = lambda tmpdir: tmpdir  # no artifact bucket here

from concourse import bass, mybir, tile, bacc
from concourse.bass_utils import run_bass_kernel_spmd

# --- problem constants ------------------------------------------------------
N, C, W_DIM, RANK, E = 100000, 128, 512, 10, 640000
NCORES = 8
NLOC = N // NCORES            # 12500
P = 128
NTILE = 98                    # ceil(12500/128)
NPAD = NTILE * P              # 12544
NSG = 25                      # super-tiles of <=512 nodes (last has 256 cols)
NBANK = 4
BROWS = N // NBANK            # 25000 rows per z bank
GBATCH = 1024                 # rows per dma_gather (hw ring limit ~1024)
NSWQ = 4                      # SWDGE queues (Q7 cpu pairs)
NEG_SLOPE = 0.01
INV_SQRT_RANK = np.float32(1.0 / np.sqrt(RANK))

f32 = mybir.dt.float32
f16 = mybir.dt.float16
f8 = mybir.dt.float8e4
i16 = mybir.dt.int16

LAST_EXEC_TIME_NS = None


def _prep_weight(w, affW, affb, W):
    """Host float32 mirror of the reference SynthesisLayer weight path."""
    styles = (w @ affW.T + affb)[0]
    L = styles[: C * RANK].reshape(C, RANK)
    R = styles[C * RANK:].reshape(RANK, C)
    mod = (L @ R) * INV_SQRT_RANK
    Wm = W * (mod + np.float32(1.0))
    Wm = Wm / (np.linalg.norm(Wm, axis=1, keepdims=True) + np.float32(1e-8))
    return Wm.astype(np.float32)


def _edge_plan(edge_index):
    """Host edge preprocessing.

    Sections are (super-tile s, bank b); edges sorted by destination within a
    section; section slot counts padded to a common multiple of 128 across
    cores. Per chunk of 128 slots, a staircase matrix maps edge slot ->
    destination column within the super-tile (zero rows for pad slots).

    Returns:
      M[s][b]      chunk count per section (shared across cores)
      CB[b]        total slots per bank stream
      idx_arrays   per core: int16 [16, sum(CB)//16] wrapped rows (banks
                   concatenated; replicated to 128 partitions on-device)
      stair        per core: fp8 [128, sw_total] concatenated staircases
      sspan        chunk -> (stair col offset, c0, w) per (s, b, j), shared
                   across cores (spans padded to the per-chunk max over cores)
    """
    src, dst = edge_index[0].astype(np.int64), edge_index[1].astype(np.int64)
    core = dst // NLOC
    d_loc = dst - core * NLOC
    sg_all = d_loc // 512
    pos_all = d_loc % 512                     # position within super-tile
    bank_all = src // BROWS
    row_all = src % BROWS

    counts = np.zeros((NCORES, NSG, NBANK), np.int64)
    np.add.at(counts, (core, sg_all, bank_all), 1)
    M = np.ceil(counts.max(axis=0) / P).astype(np.int64)    # [NSG, NBANK]
    CB = M.sum(axis=0) * P

    # sort by (core, super-tile, bank, position) so each section is
    # destination-sorted
    order = np.lexsort((pos_all, bank_all, sg_all, core))
    so_row = row_all[order]
    so_pos = pos_all[order]
    starts = np.zeros((NCORES, NSG, NBANK), np.int64)
    np.cumsum(counts.reshape(-1)[:-1], out=starts.reshape(-1)[1:])

    # section start position (slots) within each bank stream
    sec_pos = np.zeros((NSG, NBANK), np.int64)
    for b in range(NBANK):
        acc = 0
        for s in range(NSG):
            sec_pos[s, b] = acc
            acc += M[s, b] * P

    boff = np.zeros(NBANK + 1, np.int64)
    np.cumsum(CB, out=boff[1:])

    rows_all = []
    poss_all = []
    for c in range(NCORES):
        rows = np.zeros(CB.sum(), np.int64)
        poss = np.full(CB.sum(), -1, np.int64)
        for b in range(NBANK):
            for s in range(NSG):
                n = counts[c, s, b]
                st = starts[c, s, b]
                p0 = boff[b] + sec_pos[s, b]
                rows[p0: p0 + n] = so_row[st: st + n]
                poss[p0: p0 + n] = so_pos[st: st + n]
        rows_all.append(rows)
        poss_all.append(poss)

    # chunk spans: c0/w shared across cores (max span over cores); the main
    # matmul initializes the full accumulator (start=True), so every chunk
    # uses its minimal span
    sspan = {}
    sw_total = 0
    for s in range(NSG):
        for b in range(NBANK):
            for j in range(int(M[s, b])):
                c0s, c1s = [], []
                for c in range(NCORES):
                    p0 = int(boff[b]) + int(sec_pos[s, b]) + j * P
                    pp = poss_all[c][p0: p0 + P]
                    pp = pp[pp >= 0]
                    if len(pp):
                        c0s.append(int(pp.min()))
                        c1s.append(int(pp.max()))
                if not c0s:
                    c0, w = 0, 2
                else:
                    c0 = min(c0s)
                    w = max(c1s) - c0 + 1
                    w = w + (w & 1)
                    if c0 + w > 512:
                        c0 = 512 - w
                sspan[(s, b, j)] = (sw_total, c0, w)
                sw_total += w

    idx_arrays, stair_arrays = [], []
    for c in range(NCORES):
        rows = rows_all[c]
        poss = poss_all[c]
        wrapped = rows.reshape(-1, 16).T.astype(np.int16)   # [16, sum(CB)/16]
        # replicated to 128 partitions host-side: the extra HBM bytes hide
        # under the gather desc-gen chain, while an on-device replication
        # chain would delay the first gather by ~20us
        idx_arrays.append(np.ascontiguousarray(np.tile(wrapped, (8, 1))))

        stair = np.zeros((P, sw_total), F8)
        for b in range(NBANK):
            for s in range(NSG):
                for j in range(int(M[s, b])):
                    off, c0, w = sspan[(s, b, j)]
                    p0 = int(boff[b]) + int(sec_pos[s, b]) + j * P
                    pp = poss[p0: p0 + P]
                    val = pp >= 0
                    stair[val, off + (pp[val] - c0)] = F8(1.0)
        stair_arrays.append(np.ascontiguousarray(stair))

    return M, CB, idx_arrays, stair_arrays, sspan, sw_total


def _build_program(M, CB, sspan, sw_total, s_n1, s_n23):
    """Build the SPMD Bass program (section chunk counts M baked in).

    s_n1 / s_n23 are the on-device descale factors for the fp8 noise streams.
    """
    nc = bacc.Bacc("TRN2", target_bir_lowering=False, num_swdge_queues=NSWQ)

    XQ = int(CB.sum()) // 16              # idx columns
    d_idx = nc.dram_tensor("idx", [P, XQ], i16, kind="ExternalInput")
    d_banks = [
        nc.dram_tensor(f"z{b}", [BROWS, C], f16, kind="ExternalInput")
        for b in range(NBANK)
    ]
    # byte-packed per-super-tile streams: x (fp16) | n1 (fp8) | n23 (fp8)
    SB = 4 * NPAD                          # total stream bytes per partition
    d_str = nc.dram_tensor("str", [P, SB], f8, kind="ExternalInput")
    d_stair = nc.dram_tensor("stair", [P, sw_total], f8, kind="ExternalInput")
    d_wm = nc.dram_tensor("wm", [P, 2 * P], f16, kind="ExternalInput")
    d_vec = nc.dram_tensor("vec", [P, 1], f32, kind="ExternalInput")
    d_yT = nc.dram_tensor("yT", [P, NPAD], f16, kind="ExternalOutput")

    Mi = [[int(M[s, b]) for b in range(NBANK)] for s in range(NSG)]
    spos = np.zeros((NSG, NBANK), np.int64)
    for b in range(NBANK):
        acc = 0
        for s in range(NSG):
            spos[s, b] = acc
            acc += Mi[s][b]
    bank_qoff = np.zeros(NBANK + 1, np.int64)
    np.cumsum([int(CB[b]) // 16 for b in range(NBANK)], out=bank_qoff[1:])

    # super-tile geometry
    sg_w = [min(4, NTILE - 4 * s) * P for s in range(NSG)]     # 512 or 256
    str_off = np.zeros(NSG + 1, np.int64)
    for s in range(NSG):
        str_off[s + 1] = str_off[s] + 4 * sg_w[s]

    # per super-tile staircase column ranges
    st_off = np.zeros(NSG + 1, np.int64)
    for s in range(NSG):
        wsum = 0
        for b in range(NBANK):
            for j in range(Mi[s][b]):
                wsum += sspan[(s, b, j)][2]
        st_off[s + 1] = st_off[s] + wsum
    stair_w = [int(st_off[s + 1] - st_off[s]) for s in range(NSG)]
    stair_wmax = max(max(stair_w), 2)

    with tile.TileContext(nc) as tc:
        with (
            tc.tile_pool(name="const", bufs=1) as cpool,
            tc.tile_pool(name="stream", bufs=4) as spool,
            tc.tile_pool(name="work", bufs=4) as wpool,
            tc.tile_pool(name="gpool", bufs=12) as gpool,
            tc.tile_pool(name="stpool", bufs=4) as stpool,
            tc.tile_pool(name="ps1p", bufs=2, space="PSUM") as ps1pool,
            tc.tile_pool(name="ps2p", bufs=6, space="PSUM") as ps2pool,
        ):
            # idx table first: the gather stream depends on it. Each bank's
            # first-batch columns load as their own small DMA so the first
            # gathers are not gated on the full table transfer.
            t_idx = cpool.tile([P, XQ], i16, tag="idx")
            qb = GBATCH // 16
            for b in range(NBANK):
                q0 = int(bank_qoff[b])
                nc.sync.dma_start(t_idx[:, q0: q0 + qb], d_idx[:, q0: q0 + qb])
            for b in range(NBANK):
                q0 = int(bank_qoff[b])
                q1 = int(bank_qoff[b + 1])
                nc.sync.dma_start(t_idx[:, q0 + qb: q1], d_idx[:, q0 + qb: q1])

            t_wm = cpool.tile([P, 2 * P], f16)
            nc.sync.dma_start(t_wm[:], d_wm[:])
            t_vec = cpool.tile([P, 1], f32)
            nc.sync.dma_start(t_vec[:], d_vec[:])

            g_tiles = [dict() for _ in range(NBANK)]
            next_batch = [0] * NBANK
            qctr = [0]
            nbatch_tot = [
                (int(CB[b]) + GBATCH - 1) // GBATCH for b in range(NBANK)
            ]
            def ensure_gathered(b, upto_chunk, lookahead=0):
                need = (upto_chunk + GBATCH // P - 1) // (GBATCH // P)
                want = min(need + lookahead, nbatch_tot[b])
                while next_batch[b] < want:
                    g = next_batch[b]
                    lo = g * GBATCH
                    hi = min(lo + GBATCH, int(CB[b]))
                    n = hi - lo
                    t_g = gpool.tile([P, GBATCH // P, C], f16, tag=f"g{b}")
                    nc.gpsimd.dma_gather(
                        out_ap=t_g[:, : n // P, :],
                        in_ap=d_banks[b][:],
                        idxs_ap=t_idx[
                            :, int(bank_qoff[b]) + lo // 16:
                            int(bank_qoff[b]) + hi // 16
                        ],
                        num_idxs=n,
                        num_idxs_reg=n,
                        elem_size=C,
                        queue_num=qctr[0] % NSWQ,
                    )
                    qctr[0] += 1
                    g_tiles[b][g] = t_g
                    if g - 11 in g_tiles[b]:
                        del g_tiles[b][g - 11]
                    next_batch[b] = g + 1

            for b in range(NBANK):
                ensure_gathered(b, 1, lookahead=0)

            # software-pipelined emission: mm1(s) is issued one iteration
            # ahead of mm2(s)+chunks(s) so the PE never waits on h1
            state = {}

            dma_state = {}

            def emit_dma(s):
                w = sg_w[s]
                so = int(str_off[s])
                t_str = spool.tile([P, 2048], f8, tag="str")
                nc.sync.dma_start(t_str[:, : 4 * w], d_str[:, so: so + 4 * w])
                t_st = stpool.tile([P, stair_wmax], f8, tag="stair")
                if stair_w[s] > 0:
                    nc.sync.dma_start(
                        t_st[:, : stair_w[s]],
                        d_stair[:, int(st_off[s]): int(st_off[s + 1])],
                    )
                dma_state[s] = (t_str, t_st)

            def emit_front(s):
                w = sg_w[s]
                t_str, t_st = dma_state.pop(s)
                t_x = t_str[:, : 2 * w].bitcast(f16)           # [P, w] fp16
                ps1 = ps1pool.tile([P, 512], f32, tag="mm1")
                nc.tensor.matmul(ps1[:, :w], t_wm[:, 0:P], t_x,
                                 start=True, stop=True)
                t_l1 = wpool.tile([P, 512], f16, tag="l1")
                nc.scalar.activation(t_l1[:, :w], ps1[:, :w],
                                     mybir.ActivationFunctionType.Lrelu,
                                     bias=t_vec[:, 0:1], scale=1.0,
                                     alpha=NEG_SLOPE)
                t_h1 = wpool.tile([P, 512], f16, tag="h1")
                nc.vector.scalar_tensor_tensor(
                    out=t_h1[:, :w], in0=t_str[:, 2 * w: 3 * w], scalar=s_n1,
                    in1=t_l1[:, :w],
                    op0=mybir.AluOpType.mult, op1=mybir.AluOpType.add)
                state[s] = (t_str, t_st, t_h1)

            def emit_back(s):
                w = sg_w[s]
                t_str, t_st, t_h1 = state.pop(s)
                nch = sum(Mi[s])
                ps2 = ps2pool.tile([P, 512], f32, tag="mm2")
                nc.tensor.matmul(ps2[:, :w], t_wm[:, P: 2 * P], t_h1[:, :w],
                                 start=True, stop=(nch == 0),
                                 skip_group_check=True)
                seen = 0
                for b in range(NBANK):
                    for j in range(Mi[s][b]):
                        cpos = int(spos[s, b]) + j
                        ensure_gathered(b, cpos + 1)
                        gt = g_tiles[b][cpos // (GBATCH // P)]
                        gcol = cpos % (GBATCH // P)
                        off, c0, wk = sspan[(s, b, j)]
                        loff = int(off - st_off[s])
                        seen += 1
                        nc.tensor.matmul(
                            ps2[:, c0: c0 + wk],
                            gt[:, gcol, 0:C],
                            t_st[:, loff: loff + wk],
                            start=False, stop=(seen == nch),
                            skip_group_check=True)

                # final: yT = lrelu(ps2 + s_n23 * n23)
                t_s = wpool.tile([P, 512], f16, tag="s")
                nc.vector.scalar_tensor_tensor(
                    out=t_s[:, :w], in0=t_str[:, 3 * w: 4 * w], scalar=s_n23,
                    in1=ps2[:, :w],
                    op0=mybir.AluOpType.mult, op1=mybir.AluOpType.add)
                t_y = wpool.tile([P, 512], f16, tag="y")
                nc.scalar.activation(t_y[:, :w], t_s[:, :w],
                                     mybir.ActivationFunctionType.Lrelu,
                                     bias=0.0, scale=1.0, alpha=NEG_SLOPE)
                nc.scalar.dma_start(
                    d_yT[:, bass.ds(4 * P * s, w)], t_y[:, :w])

            for s in range(NSG + 2):
                if s < NSG:
                    emit_dma(s)
                if 1 <= s <= NSG:
                    emit_front(s - 1)
                if s >= 2:
                    emit_back(s - 2)

    nc.compile()
    return nc


def kernel(**inputs):
    global LAST_EXEC_TIME_NS
    inp = {k: np.asarray(v) for k, v in inputs.items()}

    w = inp["w"].astype(np.float32)
    Wm1 = _prep_weight(w, inp["lin1_affW"], inp["lin1_affb"], inp["lin1_W"])
    Wm2 = _prep_weight(w, inp["lin2_affW"], inp["lin2_affb"], inp["lin2_W"])
    Wm3 = _prep_weight(w, inp["el2_affW"], inp["el2_affb"], inp["el2_W"])

    wm = np.concatenate([Wm1.T, Wm2.T], axis=1)           # [128, 256] lhsT
    wm = np.ascontiguousarray(wm.astype(np.float16))
    vec = inp["lin1_b"].astype(np.float32).reshape(P, 1)  # [128, 1] bias1

    # fold Wm3 into the edge weights: gathered rows are z = el_W @ Wm3.T
    z = (inp["el_W"].astype(np.float32) @ Wm3.T).astype(np.float16)
    banks = [
        np.ascontiguousarray(z[b * BROWS: (b + 1) * BROWS])
        for b in range(NBANK)
    ]

    # per-node constants for the final add (everything but the matmuls):
    # ns2*noise2 + ns3*noise3 + (lin2_b + el2_b + el_b @ Wm3.T)
    cvec = (
        inp["lin2_b"] + inp["el2_b"]
        + inp["el_b"].astype(np.float32) @ Wm3.T
    ).astype(np.float32)
    n23 = (
        np.float32(inp["lin2_ns"]) * inp["lin2_noise"].astype(np.float32)
        + np.float32(inp["el2_ns"]) * inp["el2_noise"].astype(np.float32)
        + cvec[None, :]
    )
    a23 = np.float32(192.0 / max(np.abs(n23).max(), 1e-30))
    n1f = inp["lin1_noise"].astype(np.float32)
    a1 = np.float32(192.0 / max(np.abs(n1f).max(), 1e-30))

    M, CB, idx_arrays, stair_arrays, sspan, sw_total = _edge_plan(
        inp["edge_index"]
    )
    nc = _build_program(
        M, CB, sspan, sw_total,
        float(inp["lin1_ns"]) / float(a1), 1.0 / float(a23),
    )

    sg_w = [min(4, NTILE - 4 * s) * P for s in range(NSG)]

    def pack_stream(xv, n1v, n23v, c):
        """Byte-pack per-core transposed streams: x fp16 | n1 fp8 | n23 fp8."""
        sl = slice(c * NLOC, (c + 1) * NLOC)
        xT = np.zeros((P, NPAD), np.float16)
        xT[:, :NLOC] = xv[sl].astype(np.float32).T.astype(np.float16)
        n1T = np.zeros((P, NPAD), F8)
        n1T[:, :NLOC] = (n1v[sl] * a1).T.astype(F8)
        n23T = np.zeros((P, NPAD), F8)
        n23T[:, :NLOC] = (n23v[sl] * a23).T.astype(F8)
        out = np.empty((P, 4 * NPAD), np.uint8)
        off = 0
        for s in range(NSG):
            w_ = sg_w[s]
            lo = 4 * P * s
            xb = np.ascontiguousarray(xT[:, lo: lo + w_]).view(np.uint8)
            out[:, off: off + 2 * w_] = xb
            out[:, off + 2 * w_: off + 3 * w_] = (
                np.ascontiguousarray(n1T[:, lo: lo + w_]).view(np.uint8))
            out[:, off + 3 * w_: off + 4 * w_] = (
                np.ascontiguousarray(n23T[:, lo: lo + w_]).view(np.uint8))
            off += 4 * w_
        return out.view(F8)

    in_maps = []
    for c in range(NCORES):
        m = {
            "str": pack_stream(inp["x"], n1f, n23, c),
            "wm": wm, "vec": vec,
            "stair": stair_arrays[c],
            "idx": idx_arrays[c],
        }
        for b in range(NBANK):
            m[f"z{b}"] = banks[b]
        in_maps.append(m)

    trace = bool(os.environ.get("KERNEL_TRACE"))
    res = run_bass_kernel_spmd(
        nc, in_maps, core_ids=list(range(NCORES)), trace=trace
    )
    LAST_EXEC_TIME_NS = res.exec_time_ns

    y = np.empty((N, C), np.float32)
    for c in range(NCORES):
        y[c * NLOC: (c + 1) * NLOC] = (
            res.results[c]["yT"][:, :NLOC].astype(np.float32).T
        )
    return y


# revision 29
# speedup vs baseline: 1.0310x; 1.0310x over previous
"""Trainium2 Bass kernel for nn_BiasBlock (gnn_message_passing).

Computes, for N=100k nodes / E=640k edges / C=128 channels:
    h  = synth1(x)   -> synth2(h)            (modulated linears, LeakyReLU/identity)
    agg = segment_sum(el_W[src], dst) + el_b -> synth3(agg)
    y  = leaky_relu(h + agg, 0.01)

Strategy: shard nodes across 8 NeuronCores (12500 each, padded to 12544).
Per core, activations live transposed ([channel, node]). Host folds the third
modulated weight into the edge weights (z = el_W @ Wm3.T), so gathered edge
rows accumulate straight into the main PSUM accumulator: per 512-node
super-tile, ps = Wm2 @ h1 (start=True) then one staircase matmul per 128-edge
chunk (start=False) adds the whole edge branch. All per-node constants
(lin2/el2 noise, biases, el_b @ Wm3.T) are merged host-side into one fp8
stream; x stays fp16; the x/noise streams are byte-packed so each super-tile
needs a single DMA. Staircase matrices ship as fp8 (exact 0/1; matmul takes
mixed fp16 lhsT x fp8 rhs), halving their traffic. Gather indices ship
host-replicated ([128, n/16] int16) with each bank's first-batch columns as
their own small DMA so the first gathers start ~10us earlier. Emission is
software-pipelined three stages deep (stream/stair DMA 2 super-tiles ahead,
mm1 1 ahead) so neither the PE queue nor the gather queues ever stall on the
x-branch. The SWDGE descriptor-generation chain (4 queues, ~1.5us fixed +
~7.3ns/row per 1024-row gather per queue) is the kernel's critical path.

Edge branch: edges are grouped per (super-tile of 512 destination nodes) x
(z bank of 25000 rows; 4 banks keep gather indices within int16), sorted by
destination within each section, and padded to chunks of 128. z rows are
fetched in fp16 (256B) with batched gpsimd.dma_gather (1024 rows/instruction)
round-robined over all 4 SWDGE queues. Each 128-edge chunk is segment-summed
into its super-tile's [128, 512] PSUM accumulator by ONE matmul against a
static staircase matrix (edge slot -> destination column, built on the host).
"""
import os
import sys
import types

import numpy as np
import ml_dtypes

F8 = ml_dtypes.float8_e4m3  # TRN fp8e4 (E4M3 with inf, max normal 240)

# --- environment bootstrap (self-contained: no sibling imports) -------------
if "/opt/trn_rl_repo" not in sys.path:
    sys.path.insert(0, "/opt/trn_rl_repo")

_hook = {"h": None}


def _install_axon_hooks():
    """Provide antenv.axon_hooks (absent in this image) so trace=True works."""
    try:
        import antenv
    except ImportError:
        return
    if "antenv.axon_hooks" in sys.modules:
        return
    mod = types.ModuleType("antenv.axon_hooks")
    mod.set_axon_ntff_profile_hook = lambda h: _hook.__setitem__("h", h)
    mod.get_axon_ntff_profile_hook = lambda: _hook["h"]
    sys.modules["antenv.axon_hooks"] = mod
    antenv.axon_hooks = mod
    try:
        from trn_agent_boot.trn_boot import _ntff_profile_via_ctypes

        mod.set_axon_ntff_profile_hook(
            _ntff_profile_via_ctypes("/opt/axon/libaxon_pjrt.so")
        )
    except Exception:
        pass


_install_axon_hooks()

import concourse.bass_utils as _bu

_bu.upload_artifacts = lambda tmpdir: tmpdir  # no artifact bucket here

from concourse import bass, mybir, tile, bacc
from concourse.bass_utils import run_bass_kernel_spmd

# --- problem constants ------------------------------------------------------
N, C, W_DIM, RANK, E = 100000, 128, 512, 10, 640000
NCORES = 8
NLOC = N // NCORES            # 12500
P = 128
NTILE = 98                    # ceil(12500/128)
NPAD = NTILE * P              # 12544
NSG = 25                      # super-tiles of <=512 nodes (last has 256 cols)
NBANK = 4
BROWS = N // NBANK            # 25000 rows per z bank
GBATCH = 1024                 # rows per dma_gather (hw ring limit ~1024)
NSWQ = 4                      # SWDGE queues (Q7 cpu pairs)
NEG_SLOPE = 0.01
INV_SQRT_RANK = np.float32(1.0 / np.sqrt(RANK))

f32 = mybir.dt.float32
f16 = mybir.dt.float16
f8 = mybir.dt.float8e4
i16 = mybir.dt.int16

LAST_EXEC_TIME_NS = None


def _prep_weight(w, affW, affb, W):
    """Host float32 mirror of the reference SynthesisLayer weight path."""
    styles = (w @ affW.T + affb)[0]
    L = styles[: C * RANK].reshape(C, RANK)
    R = styles[C * RANK:].reshape(RANK, C)
    mod = (L @ R) * INV_SQRT_RANK
    Wm = W * (mod + np.float32(1.0))
    Wm = Wm / (np.linalg.norm(Wm, axis=1, keepdims=True) + np.float32(1e-8))
    return Wm.astype(np.float32)


def _edge_plan(edge_index):
    """Host edge preprocessing.

    Sections are (super-tile s, bank b); edges sorted by destination within a
    section; section slot counts padded to a common multiple of 128 across
    cores. Per chunk of 128 slots, a staircase matrix maps edge slot ->
    destination column within the super-tile (zero rows for pad slots).

    Returns:
      M[s][b]      chunk count per section (shared across cores)
      CB[b]        total slots per bank stream
      idx_arrays   per core: int16 [16, sum(CB)//16] wrapped rows (banks
                   concatenated; replicated to 128 partitions on-device)
      stair        per core: fp8 [128, sw_total] concatenated staircases
      sspan        chunk -> (stair col offset, c0, w) per (s, b, j), shared
                   across cores (spans padded to the per-chunk max over cores)
    """
    src, dst = edge_index[0].astype(np.int64), edge_index[1].astype(np.int64)
    core = dst // NLOC
    d_loc = dst - core * NLOC
    sg_all = d_loc // 512
    pos_all = d_loc % 512                     # position within super-tile
    bank_all = src // BROWS
    row_all = src % BROWS

    counts = np.zeros((NCORES, NSG, NBANK), np.int64)
    np.add.at(counts, (core, sg_all, bank_all), 1)
    M = np.ceil(counts.max(axis=0) / P).astype(np.int64)    # [NSG, NBANK]
    CB = M.sum(axis=0) * P

    # sort by (core, super-tile, bank, position) so each section is
    # destination-sorted
    order = np.lexsort((pos_all, bank_all, sg_all, core))
    so_row = row_all[order]
    so_pos = pos_all[order]
    starts = np.zeros((NCORES, NSG, NBANK), np.int64)
    np.cumsum(counts.reshape(-1)[:-1], out=starts.reshape(-1)[1:])

    # section start position (slots) within each bank stream
    sec_pos = np.zeros((NSG, NBANK), np.int64)
    for b in range(NBANK):
        acc = 0
        for s in range(NSG):
            sec_pos[s, b] = acc
            acc += M[s, b] * P

    boff = np.zeros(NBANK + 1, np.int64)
    np.cumsum(CB, out=boff[1:])

    rows_all = []
    poss_all = []
    for c in range(NCORES):
        rows = np.zeros(CB.sum(), np.int64)
        poss = np.full(CB.sum(), -1, np.int64)
        for b in range(NBANK):
            for s in range(NSG):
                n = counts[c, s, b]
                st = starts[c, s, b]
                p0 = boff[b] + sec_pos[s, b]
                rows[p0: p0 + n] = so_row[st: st + n]
                poss[p0: p0 + n] = so_pos[st: st + n]
        rows_all.append(rows)
        poss_all.append(poss)

    # chunk spans: c0/w shared across cores (max span over cores); the main
    # matmul initializes the full accumulator (start=True), so every chunk
    # uses its minimal span
    sspan = {}
    sw_total = 0
    for s in range(NSG):
        for b in range(NBANK):
            for j in range(int(M[s, b])):
                c0s, c1s = [], []
                for c in range(NCORES):
                    p0 = int(boff[b]) + int(sec_pos[s, b]) + j * P
                    pp = poss_all[c][p0: p0 + P]
                    pp = pp[pp >= 0]
                    if len(pp):
                        c0s.append(int(pp.min()))
                        c1s.append(int(pp.max()))
                if not c0s:
                    c0, w = 0, 2
                else:
                    c0 = min(c0s)
                    w = max(c1s) - c0 + 1
                    w = w + (w & 1)
                    if c0 + w > 512:
                        c0 = 512 - w
                sspan[(s, b, j)] = (sw_total, c0, w)
                sw_total += w

    idx_arrays, stair_arrays = [], []
    for c in range(NCORES):
        rows = rows_all[c]
        poss = poss_all[c]
        wrapped = rows.reshape(-1, 16).T.astype(np.int16)   # [16, sum(CB)/16]
        # replicated to 128 partitions host-side: the extra HBM bytes hide
        # under the gather desc-gen chain, while an on-device replication
        # chain would delay the first gather by ~20us
        idx_arrays.append(np.ascontiguousarray(np.tile(wrapped, (8, 1))))

        stair = np.zeros((P, sw_total), F8)
        for b in range(NBANK):
            for s in range(NSG):
                for j in range(int(M[s, b])):
                    off, c0, w = sspan[(s, b, j)]
                    p0 = int(boff[b]) + int(sec_pos[s, b]) + j * P
                    pp = poss[p0: p0 + P]
                    val = pp >= 0
                    stair[val, off + (pp[val] - c0)] = F8(1.0)
        stair_arrays.append(np.ascontiguousarray(stair))

    return M, CB, idx_arrays, stair_arrays, sspan, sw_total


def _build_program(M, CB, sspan, sw_total, s_n1, s_n23):
    """Build the SPMD Bass program (section chunk counts M baked in).

    s_n1 / s_n23 are the on-device descale factors for the fp8 noise streams.
    """
    nc = bacc.Bacc("TRN2", target_bir_lowering=False, num_swdge_queues=NSWQ)

    XQ = int(CB.sum()) // 16              # idx columns
    d_idx = nc.dram_tensor("idx", [P, XQ], i16, kind="ExternalInput")
    d_banks = [
        nc.dram_tensor(f"z{b}", [BROWS, C], f16, kind="ExternalInput")
        for b in range(NBANK)
    ]
    # byte-packed per-super-tile streams: x (fp16) | n1 (fp8) | n23 (fp8)
    SB = 4 * NPAD                          # total stream bytes per partition
    d_str = nc.dram_tensor("str", [P, SB], f8, kind="ExternalInput")
    d_stair = nc.dram_tensor("stair", [P, sw_total], f8, kind="ExternalInput")
    d_wm = nc.dram_tensor("wm", [P, 2 * P], f16, kind="ExternalInput")
    d_vec = nc.dram_tensor("vec", [P, 1], f32, kind="ExternalInput")
    d_yT = nc.dram_tensor("yT", [P, NPAD], f16, kind="ExternalOutput")

    Mi = [[int(M[s, b]) for b in range(NBANK)] for s in range(NSG)]
    spos = np.zeros((NSG, NBANK), np.int64)
    for b in range(NBANK):
        acc = 0
        for s in range(NSG):
            spos[s, b] = acc
            acc += Mi[s][b]
    bank_qoff = np.zeros(NBANK + 1, np.int64)
    np.cumsum([int(CB[b]) // 16 for b in range(NBANK)], out=bank_qoff[1:])

    # super-tile geometry
    sg_w = [min(4, NTILE - 4 * s) * P for s in range(NSG)]     # 512 or 256
    str_off = np.zeros(NSG + 1, np.int64)
    for s in range(NSG):
        str_off[s + 1] = str_off[s] + 4 * sg_w[s]

    # per super-tile staircase column ranges
    st_off = np.zeros(NSG + 1, np.int64)
    for s in range(NSG):
        wsum = 0
        for b in range(NBANK):
            for j in range(Mi[s][b]):
                wsum += sspan[(s, b, j)][2]
        st_off[s + 1] = st_off[s] + wsum
    stair_w = [int(st_off[s + 1] - st_off[s]) for s in range(NSG)]
    stair_wmax = max(max(stair_w), 2)

    with tile.TileContext(nc) as tc:
        with (
            tc.tile_pool(name="const", bufs=1) as cpool,
            tc.tile_pool(name="stream", bufs=4) as spool,
            tc.tile_pool(name="work", bufs=4) as wpool,
            tc.tile_pool(name="gpool", bufs=10) as gpool,
            tc.tile_pool(name="stpool", bufs=4) as stpool,
            tc.tile_pool(name="ps1p", bufs=2, space="PSUM") as ps1pool,
            tc.tile_pool(name="ps2p", bufs=4, space="PSUM") as ps2pool,
        ):
            # idx table first: the gather stream depends on it. Each bank's
            # first-batch columns load as their own small DMA so the first
            # gathers are not gated on the full table transfer.
            t_idx = cpool.tile([P, XQ], i16, tag="idx")
            qb = GBATCH // 16
            for b in range(NBANK):
                q0 = int(bank_qoff[b])
                nc.sync.dma_start(t_idx[:, q0: q0 + qb], d_idx[:, q0: q0 + qb])
            for b in range(NBANK):
                q0 = int(bank_qoff[b])
                q1 = int(bank_qoff[b + 1])
                nc.sync.dma_start(t_idx[:, q0 + qb: q1], d_idx[:, q0 + qb: q1])

            t_wm = cpool.tile([P, 2 * P], f16)
            nc.sync.dma_start(t_wm[:], d_wm[:])
            t_vec = cpool.tile([P, 1], f32)
            nc.sync.dma_start(t_vec[:], d_vec[:])

            g_tiles = [dict() for _ in range(NBANK)]
            next_batch = [0] * NBANK
            qctr = [0]
            nbatch_tot = [
                (int(CB[b]) + GBATCH - 1) // GBATCH for b in range(NBANK)
            ]
            def ensure_gathered(b, upto_chunk, lookahead=0):
                need = (upto_chunk + GBATCH // P - 1) // (GBATCH // P)
                want = min(need + lookahead, nbatch_tot[b])
                while next_batch[b] < want:
                    g = next_batch[b]
                    lo = g * GBATCH
                    hi = min(lo + GBATCH, int(CB[b]))
                    n = hi - lo
                    t_g = gpool.tile([P, GBATCH // P, C], f16, tag=f"g{b}")
                    nc.gpsimd.dma_gather(
                        out_ap=t_g[:, : n // P, :],
                        in_ap=d_banks[b][:],
                        idxs_ap=t_idx[
                            :, int(bank_qoff[b]) + lo // 16:
                            int(bank_qoff[b]) + hi // 16
                        ],
                        num_idxs=n,
                        num_idxs_reg=n,
                        elem_size=C,
                        queue_num=qctr[0] % NSWQ,
                    )
                    qctr[0] += 1
                    g_tiles[b][g] = t_g
                    if g - 9 in g_tiles[b]:
                        del g_tiles[b][g - 9]
                    next_batch[b] = g + 1

            for b in range(NBANK):
                ensure_gathered(b, 1, lookahead=0)

            # software-pipelined emission: mm1(s) is issued one iteration
            # ahead of mm2(s)+chunks(s) so the PE never waits on h1
            state = {}

            dma_state = {}

            def emit_dma(s):
                w = sg_w[s]
                so = int(str_off[s])
                t_str = spool.tile([P, 2048], f8, tag="str")
                nc.sync.dma_start(t_str[:, : 4 * w], d_str[:, so: so + 4 * w])
                t_st = stpool.tile([P, stair_wmax], f8, tag="stair")
                if stair_w[s] > 0:
                    nc.sync.dma_start(
                        t_st[:, : stair_w[s]],
                        d_stair[:, int(st_off[s]): int(st_off[s + 1])],
                    )
                dma_state[s] = (t_str, t_st)

            def emit_front(s):
                w = sg_w[s]
                t_str, t_st = dma_state.pop(s)
                t_x = t_str[:, : 2 * w].bitcast(f16)           # [P, w] fp16
                ps1 = ps1pool.tile([P, 512], f32, tag="mm1")
                nc.tensor.matmul(ps1[:, :w], t_wm[:, 0:P], t_x,
                                 start=True, stop=True)
                t_l1 = wpool.tile([P, 512], f16, tag="l1")
                nc.scalar.activation(t_l1[:, :w], ps1[:, :w],
                                     mybir.ActivationFunctionType.Lrelu,
                                     bias=t_vec[:, 0:1], scale=1.0,
                                     alpha=NEG_SLOPE)
                t_h1 = wpool.tile([P, 512], f16, tag="h1")
                nc.vector.scalar_tensor_tensor(
                    out=t_h1[:, :w], in0=t_str[:, 2 * w: 3 * w], scalar=s_n1,
                    in1=t_l1[:, :w],
                    op0=mybir.AluOpType.mult, op1=mybir.AluOpType.add)
                state[s] = (t_str, t_st, t_h1)

            def emit_back(s):
                w = sg_w[s]
                t_str, t_st, t_h1 = state.pop(s)
                nch = sum(Mi[s])
                ps2 = ps2pool.tile([P, 512], f32, tag="mm2")
                nc.tensor.matmul(ps2[:, :w], t_wm[:, P: 2 * P], t_h1[:, :w],
                                 start=True, stop=(nch == 0),
                                 skip_group_check=True)
                seen = 0
                for b in range(NBANK):
                    for j in range(Mi[s][b]):
                        cpos = int(spos[s, b]) + j
                        ensure_gathered(b, cpos + 1)
                        gt = g_tiles[b][cpos // (GBATCH // P)]
                        gcol = cpos % (GBATCH // P)
                        off, c0, wk = sspan[(s, b, j)]
                        loff = int(off - st_off[s])
                        seen += 1
                        nc.tensor.matmul(
                            ps2[:, c0: c0 + wk],
                            gt[:, gcol, 0:C],
                            t_st[:, loff: loff + wk],
                            start=False, stop=(seen == nch),
                            skip_group_check=True)

                # final: yT = lrelu(ps2 + s_n23 * n23)
                t_s = wpool.tile([P, 512], f16, tag="s")
                nc.vector.scalar_tensor_tensor(
                    out=t_s[:, :w], in0=t_str[:, 3 * w: 4 * w], scalar=s_n23,
                    in1=ps2[:, :w],
                    op0=mybir.AluOpType.mult, op1=mybir.AluOpType.add)
                t_y = wpool.tile([P, 512], f16, tag="y")
                nc.scalar.activation(t_y[:, :w], t_s[:, :w],
                                     mybir.ActivationFunctionType.Lrelu,
                                     bias=0.0, scale=1.0, alpha=NEG_SLOPE)
                nc.scalar.dma_start(
                    d_yT[:, bass.ds(4 * P * s, w)], t_y[:, :w])

            for s in range(NSG + 2):
                if s < NSG:
                    emit_dma(s)
                if 1 <= s <= NSG:
                    emit_front(s - 1)
                if s >= 2:
                    emit_back(s - 2)

    nc.compile()
    return nc


def kernel(**inputs):
    global LAST_EXEC_TIME_NS
    inp = {k: np.asarray(v) for k, v in inputs.items()}

    w = inp["w"].astype(np.float32)
    Wm1 = _prep_weight(w, inp["lin1_affW"], inp["lin1_affb"], inp["lin1_W"])
    Wm2 = _prep_weight(w, inp["lin2_affW"], inp["lin2_affb"], inp["lin2_W"])
    Wm3 = _prep_weight(w, inp["el2_affW"], inp["el2_affb"], inp["el2_W"])

    wm = np.concatenate([Wm1.T, Wm2.T], axis=1)           # [128, 256] lhsT
    wm = np.ascontiguousarray(wm.astype(np.float16))
    vec = inp["lin1_b"].astype(np.float32).reshape(P, 1)  # [128, 1] bias1

    # fold Wm3 into the edge weights: gathered rows are z = el_W @ Wm3.T
    z = (inp["el_W"].astype(np.float32) @ Wm3.T).astype(np.float16)
    banks = [
        np.ascontiguousarray(z[b * BROWS: (b + 1) * BROWS])
        for b in range(NBANK)
    ]

    # per-node constants for the final add (everything but the matmuls):
    # ns2*noise2 + ns3*noise3 + (lin2_b + el2_b + el_b @ Wm3.T)
    cvec = (
        inp["lin2_b"] + inp["el2_b"]
        + inp["el_b"].astype(np.float32) @ Wm3.T
    ).astype(np.float32)
    n23 = (
        np.float32(inp["lin2_ns"]) * inp["lin2_noise"].astype(np.float32)
        + np.float32(inp["el2_ns"]) * inp["el2_noise"].astype(np.float32)
        + cvec[None, :]
    )
    a23 = np.float32(192.0 / max(np.abs(n23).max(), 1e-30))
    n1f = inp["lin1_noise"].astype(np.float32)
    a1 = np.float32(192.0 / max(np.abs(n1f).max(), 1e-30))

    M, CB, idx_arrays, stair_arrays, sspan, sw_total = _edge_plan(
        inp["edge_index"]
    )
    nc = _build_program(
        M, CB, sspan, sw_total,
        float(inp["lin1_ns"]) / float(a1), 1.0 / float(a23),
    )

    sg_w = [min(4, NTILE - 4 * s) * P for s in range(NSG)]

    def pack_stream(xv, n1v, n23v, c):
        """Byte-pack per-core transposed streams: x fp16 | n1 fp8 | n23 fp8."""
        sl = slice(c * NLOC, (c + 1) * NLOC)
        xT = np.zeros((P, NPAD), np.float16)
        xT[:, :NLOC] = xv[sl].astype(np.float32).T.astype(np.float16)
        n1T = np.zeros((P, NPAD), F8)
        n1T[:, :NLOC] = (n1v[sl] * a1).T.astype(F8)
        n23T = np.zeros((P, NPAD), F8)
        n23T[:, :NLOC] = (n23v[sl] * a23).T.astype(F8)
        out = np.empty((P, 4 * NPAD), np.uint8)
        off = 0
        for s in range(NSG):
            w_ = sg_w[s]
            lo = 4 * P * s
            xb = np.ascontiguousarray(xT[:, lo: lo + w_]).view(np.uint8)
            out[:, off: off + 2 * w_] = xb
            out[:, off + 2 * w_: off + 3 * w_] = (
                np.ascontiguousarray(n1T[:, lo: lo + w_]).view(np.uint8))
            out[:, off + 3 * w_: off + 4 * w_] = (
                np.ascontiguousarray(n23T[:, lo: lo + w_]).view(np.uint8))
            off += 4 * w_
        return out.view(F8)

    in_maps = []
    for c in range(NCORES):
        m = {
            "str": pack_stream(inp["x"], n1f, n23, c),
            "wm": wm, "vec": vec,
            "stair": stair_arrays[c],
            "idx": idx_arrays[c],
        }
        for b in range(NBANK):
            m[f"z{b}"] = banks[b]
        in_maps.append(m)

    trace = bool(os.environ.get("KERNEL_TRACE"))
    res = run_bass_kernel_spmd(
        nc, in_maps, core_ids=list(range(NCORES)), trace=trace
    )
    LAST_EXEC_TIME_NS = res.exec_time_ns

    y = np.empty((N, C), np.float32)
    for c in range(NCORES):
        y[c * NLOC: (c + 1) * NLOC] = (
            res.results[c]["yT"][:, :NLOC].astype(np.float32).T
        )
    return y
